# revision 53
# baseline (speedup 1.0000x reference)
"""Causal multi-head attention on 8 Trainium2 NeuronCores.

Sharding: data-parallel over batch (B=2) x tensor-parallel over heads
(16 heads -> 4 groups of 4). Core (b, hg) computes, for batch b and its
4 heads: Q/K/V projections, causal attention, and a partial output
projection against its slice of Wo. The host sums the 4 partials per
batch (the "all-reduce" of the reference TP recipe, done at unshard).

Per-core dataflow (all matmuls bf16 inputs, fp32 PSUM accumulate):
  QT = Wq_hg @ x_b.T          [256, 2048]   (head dim on partitions)
  KT = Wk_hg @ x_b.T          [256, 2048]
  V  = x_b @ Wv_hg.T          [2048, 256]   (seq on partitions)
  per (head pair, q block, 128-wide k tile):
    scoresT[k, q] = K_h @ Q_h.T      (two heads at PE row groups 0-63/64-127)
    expT = exp(scoresT / 8) * causal_mask          (ACT, bf16 out)
    outT[65, q]  += [V_h | ones].T @ expT          (row 64 = softmax denom)
  normalize via reciprocal + rank-1 fp32r PE broadcasts, then
  out_partial = attnT.T @ WoT_hg      [2048, 1024] fp32

Schedule notes (sim-profiled): DMA issue costs ~600ns/instr on a queue
and ~900ns completion latency, so inputs arrive as a few large strided
DMAs on one queue in exact need-order. The attention inner loop is
exactly PE:ACT balanced (scores+PV columns == exp columns), so all
remaining projection work (Q n>0, V st>3) plus deferred norm/Wo jobs is
dribbled into the score stream as fillers through a readiness-gated
queue; PV trails its exp by three k-tiles. The last q block is split
into two 256-wide sub-blocks to shorten the final norm->Wo->store tail.
"""

import numpy as np
import ml_dtypes

import concourse.bass as bass
import concourse.mybir as mybir
from concourse.tile import TileContext
from concourse.bass_utils import run_bass_kernel_spmd

B, S, D, H = 2, 2048, 1024, 16
NCORES, NHG = 8, 4          # cores, head groups
HL = H // NHG               # 4 heads per core
DK = D // H                 # 64
HD = HL * DK                # 256 local head dims
P = 128
KO = D // P                 # 8 contraction tiles over D
QB = 512                    # q block width
NST = S // P                # 16 seq tiles

bf16 = ml_dtypes.bfloat16
BF, F32, FR = mybir.dt.bfloat16, mybir.dt.float32, mybir.dt.float32r
EXP = mybir.ActivationFunctionType.Exp
MUL = mybir.AluOpType.mult

# q blocks: (q_start, q_width); last 512 split in two to shrink the tail
BLOCKS = [(0, QB), (QB, QB), (2 * QB, QB), (3 * QB, QB // 2),
          (3 * QB + QB // 2, QB // 2)]


def _split_multiwaits(nc, max_waits=1):
    # The walrus build in this container accepts at most one sync-wait
    # command per instruction; hoist extra waits onto single-wait NoOps
    # preceding the instruction on the same engine.
    for f in nc.m.functions:
        for bb in f.blocks:
            new = []
            changed = False
            for ins in bb.instructions:
                si = ins.sync_info
                if si is not None and si.on_wait and len(si.on_wait) > max_waits:
                    waits = list(si.on_wait)
                    for k, w in enumerate(waits[:-max_waits]):
                        new.append(mybir.InstNoOp(
                            name=f"{ins.name}-wsplit{k}",
                            engine=ins.engine,
                            sync_info=mybir.SyncInfo(on_wait=[w], on_update=[]),
                            bass_nofuse=True,
                        ))
                    si.on_wait = waits[-max_waits:]
                    changed = True
                new.append(ins)
            if changed:
                bb.instructions = new


def _build():
    nc = bass.Bass()
    xT = nc.dram_tensor("xT", [P, KO, S], BF, kind="ExternalInput")
    wq = nc.dram_tensor("wq", [P, KO, HD], BF, kind="ExternalInput")
    wk = nc.dram_tensor("wk", [P, KO, HD], BF, kind="ExternalInput")
    wv = nc.dram_tensor("wv", [P, KO, HD], BF, kind="ExternalInput")
    wo = nc.dram_tensor("wo", [P, HD // P, D], BF, kind="ExternalInput")
    masks = nc.dram_tensor("masks", [P, 2, P], BF, kind="ExternalInput")
    eye2 = nc.dram_tensor("eye2", [1, 2, P], FR, kind="ExternalInput")
    out = nc.dram_tensor("out", [S, D], F32, kind="ExternalOutput")

    with TileContext(nc) as tc:
        with (
            tc.tile_pool(name="const", bufs=1) as cp,
            tc.tile_pool(name="work", bufs=8) as wp,
            tc.tile_pool(name="rwork", bufs=2) as rp,
            tc.tile_pool(name="psS", bufs=2, space="PSUM") as psS,
            tc.tile_pool(name="psO", bufs=2, space="PSUM") as psO,
            tc.tile_pool(name="psM", bufs=2, space="PSUM") as psM,
        ):
            xT_sb = cp.tile([P, KO, S], BF, tag="xT")
            wq_sb = cp.tile([P, KO, HD], BF, tag="wq")
            wk_sb = cp.tile([P, KO, HD], BF, tag="wk")
            wv_sb = cp.tile([P, KO, HD], BF, tag="wv")
            wo_sb = cp.tile([P, HD // P, D], BF, tag="wo")
            mk_sb = cp.tile([P, 2, P], BF, tag="mk")
            eye2_sb = cp.tile([1, 2, P], FR, tag="eye2")
            # DMA issue costs ~600ns and the DMA engines drain transfers
            # serially, so ship few, large, strided transfers from ONE queue
            # in exact need-order: first Q operands in fine chunks, then
            # weights/blocks in the order the PE stream consumes them.
            for c in range(0, KO, 2):
                nc.sync.dma_start(wq_sb[:, c:c + 2], wq[:, c:c + 2])
                nc.sync.dma_start(xT_sb[:, c:c + 2, bass.ts(0, QB)],
                                  xT[:, c:c + 2, bass.ts(0, QB)])
            nc.sync.dma_start(wk_sb[:], wk[:])
            nc.sync.dma_start(xT_sb[:, :, bass.ts(1, QB)],
                              xT[:, :, bass.ts(1, QB)])
            nc.sync.dma_start(wv_sb[:], wv[:])
            nc.sync.dma_start(xT_sb[:, :, bass.ts(2, QB)],
                              xT[:, :, bass.ts(2, QB)])
            nc.sync.dma_start(mk_sb[:], masks[:])
            nc.sync.dma_start(eye2_sb[:], eye2[:])
            nc.sync.dma_start(xT_sb[:, :, bass.ts(3, QB)],
                              xT[:, :, bass.ts(3, QB)])
            nc.sync.dma_start(wo_sb[:], wo[:])

            QT_sb = cp.tile([P, HD // P, S], BF, tag="QT")
            KT_sb = cp.tile([P, HD // P, S], BF, tag="KT")
            # V with a ones column appended per head: [p, seq_tile, head, 65]
            va_sb = cp.tile([P, NST, HL, DK + 1], BF, tag="va")
            nc.vector.memset(va_sb[:, :, :, DK:DK + 1], 1.0)
            attnT_sb = cp.tile([P, HD // P, S], BF, tag="attnT")

            # prime the ACT exp table set while PE runs the projections
            warm = rp.tile([1, 8], F32, tag="warm")
            nc.vector.memset(warm[:], 0.0)
            nc.scalar.activation(warm[:], warm[:], EXP)

            # ---- projection building blocks ----
            def q_group(n, m):
                def run():
                    ns = bass.ts(n, QB)
                    pq = psM.tile([P, QB], F32, tag="ps1", name="pq")
                    for k in range(KO):
                        nc.tensor.matmul(pq[:], wq_sb[:, k, bass.ts(m, P)],
                                         xT_sb[:, k, ns],
                                         start=(k == 0), stop=(k == KO - 1))
                    nc.vector.tensor_copy(QT_sb[:, m, ns], pq[:])
                return run

            def k_group(n, m):
                def run():
                    ns = bass.ts(n, QB)
                    pk = psM.tile([P, QB], F32, tag="ps1", name="pk")
                    for k in range(KO):
                        nc.tensor.matmul(pk[:], wk_sb[:, k, bass.ts(m, P)],
                                         xT_sb[:, k, ns],
                                         start=(k == 0), stop=(k == KO - 1))
                    nc.vector.tensor_copy(KT_sb[:, m, ns], pk[:])
                return run

            def v_group(st):
                def run():
                    pv = psM.tile([P, QB], F32, tag="ps1", name="pv")
                    for k in range(KO):
                        nc.tensor.matmul(pv[:, :HD], xT_sb[:, k, bass.ts(st, P)],
                                         wv_sb[:, k],
                                         start=(k == 0), stop=(k == KO - 1))
                    nc.vector.tensor_copy(
                        va_sb[:, st, :, 0:DK],
                        pv[:, :HD].rearrange("p (h d) -> p h d", d=DK))
                return run

            # ---- deferred attention jobs ----
            def norm_a(po, poc, rc, tail=False):
                # DVE-only: 1/denominator straight from the PSUM accumulator
                # (PSUM-source ops tolerate a base-partition offset; SBUF-
                # source ones do not), then copy the accumulators out of
                # PSUM, freeing the psO slots for the next head pair. The
                # last pair skips the copies — nothing reuses its slots and
                # the multiply can read PSUM directly (shorter tail).
                def run():
                    for hh in range(2):
                        with nc.allow_low_precision(reason="fp32r is fp32-width"):
                            nc.vector.reciprocal(rc[hh][:], po[hh][DK:DK + 1])
                        if not tail:
                            nc.vector.tensor_copy(poc[hh][:], po[hh][:])
                return run

            def norm_b(q0, qw, hp, poc, rc, po=None):
                # PE part two k-tiles later (the DVE chain above has drained):
                # per head, a rank-1 fp32r matmul broadcasts the reciprocal
                # row across partitions 0..63, then the multiply reads both
                # inputs at base partition 0 (HW requires aligned operand
                # bases; only the output may sit at a 64-offset). In the tail
                # flavor (po given) the multiply reads the accumulator from
                # PSUM and the broadcast routes through the by-then-idle ACT
                # engine, keeping the final chain off the DVE queue.
                def run():
                    for hh in range(2):
                        pb = psM.tile([P, QB], F32, tag="ps1", name=f"pb{hh}")
                        nc.tensor.matmul(pb[:DK, :qw], eye2_sb[0:1, 0, 0:DK],
                                         rc[hh][:], start=True, stop=True)
                        if po is not None:
                            bc = rp.tile([DK, qw], F32, tag="bc", name="bc")
                            nc.scalar.copy(bc[:], pb[0:DK, :qw])
                            nc.vector.tensor_tensor(
                                attnT_sb[hh * DK:(hh + 1) * DK, hp,
                                         bass.ds(q0, qw)],
                                po[hh][0:DK], bc[:], MUL)
                        else:
                            nc.vector.tensor_tensor(
                                attnT_sb[hh * DK:(hh + 1) * DK, hp,
                                         bass.ds(q0, qw)],
                                poc[hh][0:DK], pb[0:DK, :qw], MUL)
                return run

            def wo_job(st, n, split=False):
                def run():
                    pw = psM.tile([P, QB], F32, tag="ps1", name="pw")
                    for i in range(HD // P):
                        nc.tensor.matmul(pw[:], attnT_sb[:, i, bass.ts(st, P)],
                                         wo_sb[:, i, bass.ts(n, QB)],
                                         start=(i == 0), stop=(i == HD // P - 1))
                    ot = wp.tile([P, QB], F32, tag="out", name="ot")
                    nc.vector.tensor_copy(ot[:], pw[:])
                    nc.sync.dma_start(out[bass.ts(st, P), bass.ts(n, QB)],
                                      ot[:])
                return run

            def pv_job(po, kt, ex, off, nkt, hp):
                def run():
                    for hh in range(2):
                        nc.tensor.matmul(po[hh][:, off:],
                                         va_sb[:, kt, 2 * hp + hh],
                                         ex[:, hh, off:],
                                         start=(kt == 0), stop=(kt == nkt - 1))
                return run

            # ---- serial head: Q/K/V for the first q block ----
            # Q(n0) with the two m-tiles interleaved per k so the PE
            # consumption rate (~427ns/k-tile) tracks the paired wq/x0
            # chunk DMAs instead of stalling on the second half
            pq = [psM.tile([P, QB], F32, tag="ps1", name=f"pq{m}")
                  for m in range(HD // P)]
            for k in range(KO):
                for m in range(HD // P):
                    nc.tensor.matmul(pq[m][:], wq_sb[:, k, bass.ts(m, P)],
                                     xT_sb[:, k, bass.ts(0, QB)],
                                     start=(k == 0), stop=(k == KO - 1))
            for m in range(HD // P):
                nc.vector.tensor_copy(QT_sb[:, m, bass.ts(0, QB)], pq[m][:])
            for m in range(HD // P):
                k_group(0, m)()
            # only V(st3) must precede att(0): PV(kt) pops 3 k-tiles after
            # its scores, so V(st0..2) popped as the first fillers land in
            # time, and attention starts ~2.5us earlier
            v_group(3)()

            # remaining projections become attention fillers (x blocks land
            # before the corresponding filler pops); normalization jobs free
            # PSUM accumulator slots, so they take a priority lane, and the
            # K-projection groups ride a middle lane gated just ahead of the
            # q block that first reads those columns
            starts = []   # global-kt index at which each block starts
            gtot = 0
            for (q0, qw) in BLOCKS:
                starts.append(gtot)
                gtot += 2 * ((q0 + qw) // P)
            urgent = []  # norm work
            midq = []    # K projections
            fillq = []   # bulk: Q/V projections and Wo jobs
            for st in range(3):
                fillq.append((0, v_group(st)))
            for n in range(1, S // QB):
                midq.append((max(0, starts[n] - 2), k_group(n, 0)))
                midq.append((max(0, starts[n] - 2), k_group(n, 1)))
                for m in range(HD // P):
                    fillq.append((0, q_group(n, m)))
                if n < 3:
                    fillq.append((0, v_group(3 + n)))
            for st in range(6, NST):
                # V(12..15) feed only the last two sub-blocks; hold them
                # back so those filler-starved stretches keep the PE busy
                gate = 0
                if st >= 14:
                    gate = starts[4] - 4
                elif st >= 12:
                    gate = starts[3] - 4
                fillq.append((gate, v_group(st)))

            pending_pv = []   # PV trails its exp by two k-tile iterations
            gk = 0

            def scan_pop(q):
                for i, (r, fn) in enumerate(q):
                    if r <= gk:
                        q.pop(i)[1]()
                        return True
                return False

            def pop_fill():
                # urgent norm work is (mostly) DVE-side; also pop one
                # PE-bearing filler so the slot still feeds the PE
                scan_pop(urgent)
                _ = scan_pop(midq) or scan_pop(fillq)

            for bi, (q0, qw) in enumerate(BLOCKS):
                nkt = (q0 + qw) // P
                for hp in range(HD // P):
                    # drain the previous pair's trailing PV (its exp is old)
                    while pending_pv:
                        pending_pv.pop(0)()
                    po = [psO.tile([DK + 1, qw], F32, tag="psO",
                                   name=f"po{q0}_{hp}_{i}") for i in range(2)]
                    for kt in range(nkt):
                        off = max(0, kt * P - q0)
                        w = qw - off
                        ex = wp.tile([P, 2, qw], BF, tag="exp")
                        # psS padded to full QB width so dim1's stride stays
                        # PSUM-bank-aligned (a [P,2,256] tile with 1KB stride
                        # wedges the device; the padded form is the proven
                        # baseline layout)
                        ps = psS.tile([P, 2, QB], F32, tag="psS")
                        for hh in range(2):
                            hsl = slice(hh * DK, (hh + 1) * DK)
                            nc.tensor.matmul(ps[:, hh, bass.ds(off, w)],
                                             KT_sb[hsl, hp, bass.ts(kt, P)],
                                             QT_sb[hsl, hp, bass.ds(q0 + off, w)],
                                             start=True, stop=True)
                        nc.scalar.activation(ex[:, :, off:],
                                             ps[:, :, bass.ds(off, w)],
                                             EXP, scale=1.0 / 8.0)
                        if kt * P >= q0:
                            # only the leading 128 remaining columns straddle
                            # the diagonal; later ones are fully visible
                            nc.vector.tensor_tensor(ex[:, :, off:off + P],
                                                    ex[:, :, off:off + P],
                                                    mk_sb[:], MUL)
                        pending_pv.append(pv_job(po, kt, ex, off, nkt, hp))
                        # keep at most 1 pending at the pair's edge (the
                        # leftover drains at the next pair's start, when its
                        # exp is comfortably old), else at most 3
                        lim = 1 if kt == nkt - 1 else 3
                        while len(pending_pv) > lim:
                            pending_pv.pop(0)()
                        pop_fill()
                        gk += 1
                    tail = (bi == len(BLOCKS) - 1 and hp == HD // P - 1)
                    poc = None if tail else [
                        rp.tile([DK + 1, qw], F32, tag=f"poc{i}",
                                name=f"poc{i}") for i in range(2)]
                    rc = [rp.tile([1, qw], FR, tag=f"rc{i}", name=f"rc{i}")
                          for i in range(2)]
                    urgent.append((gk, norm_a(po, poc, rc, tail=tail)))
                    urgent.append((gk + 3, norm_b(q0, qw, hp, poc, rc,
                                                  po=po if tail else None)))
                ready = gk + 4
                wos = [(st, n) for st in range(q0 // P, (q0 + qw) // P)
                       for n in range(D // QB)]
                for j, (st, n) in enumerate(wos):
                    # hold the last two back for the NEXT block boundary,
                    # whose first k-tiles otherwise have no PE-bearing
                    # fillers (norm_b pops at +3, new Wo jobs at +4)
                    if j >= len(wos) - 2 and bi + 2 < len(BLOCKS):
                        gate = starts[bi + 2]
                    else:
                        gate = ready + j
                    fillq.append((gate, wo_job(st, n)))
            while pending_pv:
                pending_pv.pop(0)()
            for _, job in urgent:
                job()
            for _, job in midq:
                job()
            for _, job in fillq:
                job()

    _split_multiwaits(nc)
    return nc


_NC_CACHE = []


def _prepare_in_maps(x, Wq, Wk, Wv, Wo):
    def tile_k(a, free):
        # [D, free] -> [P, KO_like, free] partition-tiled bf16
        ko = a.shape[0] // P
        return np.ascontiguousarray(
            a.reshape(ko, P, free).transpose(1, 0, 2)).astype(bf16)

    # causal triangle for the diagonal 128-col strip, duplicated for the
    # two packed heads
    tri = (np.arange(P)[:, None] <= np.arange(P)[None, :]).astype(np.float32)
    mk = np.stack([tri, tri], axis=1).astype(bf16)
    # head-pair selector: broadcast denom row hh to partitions [64hh, 64hh+64)
    eye2 = np.zeros((1, 2, P), np.float32)
    eye2[0, 0, :DK] = 1.0
    eye2[0, 1, DK:] = 1.0

    in_maps = []
    for core in range(NCORES):
        b, hg = divmod(core, NHG)
        sl = slice(hg * HD, (hg + 1) * HD)
        xb = np.asarray(x[b], np.float32)
        in_maps.append({
            "xT": tile_k(xb.T, S),
            "wq": tile_k(np.asarray(Wq[sl], np.float32).T, HD),
            "wk": tile_k(np.asarray(Wk[sl], np.float32).T, HD),
            "wv": tile_k(np.asarray(Wv[sl], np.float32).T, HD),
            "wo": tile_k(np.asarray(Wo[:, sl], np.float32).T, D),
            "masks": mk,
            "eye2": eye2,
        })
    return in_maps


def kernel(x, Wq, Wk, Wv, Wo):
    if not _NC_CACHE:
        _NC_CACHE.append(_build())
    nc = _NC_CACHE[0]
    in_maps = _prepare_in_maps(x, Wq, Wk, Wv, Wo)
    res = run_bass_kernel_spmd(nc, in_maps, core_ids=list(range(NCORES)))
    out = np.zeros((B, S, D), np.float32)
    for core in range(NCORES):
        out[core // NHG] += res.results[core]["out"]
    return out


def hw_time(inputs, iters=24):
    """Test-only helper: measure per-execution device time by issuing the
    compiled NEFF back-to-back with resident device inputs (no donation, so
    buffers are reusable) and fitting the per-iteration slope. The axon NTFF
    profiling hook isn't available in this container, so this amortized
    wall-clock slope is the closest proxy for HW exec time."""
    import time
    import jax
    from concourse import bass2jax
    import concourse.mybir as mybir_

    if not _NC_CACHE:
        _NC_CACHE.append(_build())
    nc = _NC_CACHE[0]
    in_maps = _prepare_in_maps(**inputs)

    bass2jax.install_neuronx_cc_hook()
    pid_name = nc.partition_id_tensor.name if nc.partition_id_tensor else None
    in_names, out_names, out_avals, zero_outs = [], [], [], []
    for alloc in nc.m.functions[0].allocations:
        if not isinstance(alloc, mybir_.MemoryLocationSet):
            continue
        name = alloc.memorylocations[0].name
        if alloc.kind == "ExternalInput":
            if name != pid_name:
                in_names.append(name)
        elif alloc.kind == "ExternalOutput":
            out_names.append(name)
            shape = tuple(alloc.tensor_shape)
            dtype = mybir_.dt.np(alloc.dtype)
            out_avals.append(jax.core.ShapedArray(shape, dtype))
            zero_outs.append(np.zeros(shape, dtype))
    n_params = len(in_names)
    all_names = in_names + out_names
    if pid_name is not None:
        all_names = all_names + [pid_name]

    def _body(*args):
        operands = list(args)
        if pid_name is not None:
            operands.append(bass2jax.partition_id_tensor())
        outs = bass2jax._bass_exec_p.bind(
            *operands,
            out_avals=tuple(out_avals),
            in_names=tuple(all_names),
            out_names=tuple(out_names),
            lowering_input_output_aliases=(),
            sim_require_finite=True,
            sim_require_nnan=True,
            nc=nc,
        )
        return tuple(outs)

    devices = jax.devices()[:NCORES]
    mesh = bass2jax.Mesh(np.asarray(devices), ("core",))
    spec = bass2jax.PartitionSpec("core")
    n_args = n_params + len(out_names)
    fn = jax.jit(bass2jax.shard_map(
        _body, mesh=mesh, in_specs=(spec,) * n_args,
        out_specs=(spec,) * len(out_names), check_rep=False))
    sharding = jax.sharding.NamedSharding(mesh, spec)
    concat_in = [
        jax.device_put(
            np.concatenate([np.asarray(in_maps[c][nm]) for c in range(NCORES)], axis=0),
            sharding)
        for nm in in_names
    ]
    concat_zeros = [
        jax.device_put(np.zeros((NCORES * z.shape[0], *z.shape[1:]), z.dtype), sharding)
        for z in zero_outs
    ]
    # warm up (compile + first exec)
    jax.block_until_ready(fn(*concat_in, *concat_zeros))

    def run_n(n):
        t0 = time.perf_counter()
        o = None
        for _ in range(n):
            o = fn(*concat_in, *concat_zeros)
        jax.block_until_ready(o)
        return time.perf_counter() - t0

    slopes = []
    walls = []
    for _ in range(5):
        t1 = run_n(16)
        t2 = run_n(64)
        walls.append(t2 / 64)
        slopes.append((t2 - t1) / 48)
    # min slope = least host/tunnel contention; still includes per-launch
    # runtime overhead, so it upper-bounds the true kernel span. Under a
    # heavily contended tunnel the slope fit can go nonsensical (even
    # negative); fall back to the best per-exec wall average then.
    cands = [s for s in slopes if s * 1e9 > 10000]
    if cands:
        return int(min(cands) * 1e9)
    return int(min(walls) * 1e9)
